# revision 1
# baseline (speedup 1.0000x reference)
"""Trainium2 Bass kernel for RecGRU_W_up (gnn_message_passing).

Computes, for N=24 nodes / C=4096 channels / 8 NeuronCores:
    Z = sigmoid(conv(X,Wc_xz) @ w_x_z.T + conv(H,Wc_hz) @ w_q_z.T + b_z)
    R = sigmoid(conv(X,Wc_xr) @ w_x_r.T + conv(H,Wc_hr) @ w_q_r.T + b_r)
    Ht = tanh(conv(X,Wc_xh) @ w_x_h.T + conv(H*R,Wc_hh) @ w_q_h.T + b_h)
    out = Z*Ht + (1-Z)*H
where conv(T, W) = relu(LX @ W), LX = (2/lam)(T - A_norm T) - T.

Sharding: tensor-parallel over the channel dim, fp32 everywhere.
  - conv GEMMs: weight column-sharded -> each core holds CV slice [24, 512].
  - gate GEMMs: contraction (input-dim) sharded -> each core computes a
    full-width partial [24, 4096]; r is AllReduce-summed (R is needed in
    full for H*R), z and h are ReduceScatter-summed (only the local
    column slice is needed for the elementwise combine); the host
    concatenates the 8 per-core output slices.
  - graph propagation: the rescaled-Laplacian step is folded into a
    [24,24] matrix M on the host; L(X).T / L(H).T are host-precomputed
    (0.8 MFLOP), L(H*R).T is computed on the PE mid-kernel.
"""

import numpy as np

import concourse.bass as bass
import concourse.mybir as mybir
import concourse.tile as tile
from concourse import bacc
from concourse.bass_utils import run_bass_kernel_spmd

dt = mybir.dt
AF = mybir.ActivationFunctionType
ALU = mybir.AluOpType

N = 24
C = 4096
NCORES = 8
CL = C // NCORES          # 512 channels per core
KB = C // 128             # 32 k-blocks over full C
KBL = CL // 128           # 4 k-blocks over the local slice
NSL = C // 512            # 8 output slices of 512 for full-width GEMMs
FV = (N * C) // 128       # 768: free size of the [128, .] flat view

REPLICAS = [list(range(NCORES))]

_CACHE = {}
RUN_KWARGS = {}
LAST_RESULT = None


def _flat_view(ap):
    """[24, 4096] dram tensor viewed as [128, 768] (same bytes)."""
    return ap.rearrange("n c -> (n c)").rearrange("(p f) -> p f", p=128)


def _build():
    nc = bacc.Bacc("TRN2", target_bir_lowering=False, debug=False,
                   num_devices=NCORES)

    # ---- I/O ----
    Hn = nc.dram_tensor("Hn", [N, C], dt.float32, kind="ExternalInput")
    # L(X).T and L(H).T, host-computed, already in PE tile layout [128, 32*24]
    LXT = nc.dram_tensor("LXT", [128, KB * N], dt.float32,
                         kind="ExternalInput")
    LHT = nc.dram_tensor("LHT", [128, KB * N], dt.float32,
                         kind="ExternalInput")
    Mm = nc.dram_tensor("Mm", [N, N], dt.float32, kind="ExternalInput")
    WarmI = nc.dram_tensor("WarmI", [128, 512], dt.bfloat16,
                           kind="ExternalInput")
    I24 = nc.dram_tensor("I24", [N, N], dt.float32, kind="ExternalInput")
    convW = {}
    gateU = {}
    for g in ("xz", "hz", "xr", "hr", "xh", "hh"):
        convW[g] = nc.dram_tensor(f"W{g}", [C, CL], dt.float32,
                                  kind="ExternalInput")
        gateU[g] = nc.dram_tensor(f"U{g}", [CL, C], dt.float32,
                                  kind="ExternalInput")
    Br = nc.dram_tensor("Br", [N, C], dt.float32, kind="ExternalInput")
    # per-core column slices (host pre-sliced for this core's RS chunk)
    BzS = nc.dram_tensor("BzS", [N, CL], dt.float32, kind="ExternalInput")
    BhS = nc.dram_tensor("BhS", [N, CL], dt.float32, kind="ExternalInput")
    HS = nc.dram_tensor("HS", [N, CL], dt.float32, kind="ExternalInput")
    out = nc.dram_tensor("out", [N, CL], dt.float32, kind="ExternalOutput")

    with tile.TileContext(nc) as tc:
        with (
            tc.tile_pool(name="const", bufs=1) as const,
            tc.tile_pool(name="nat", bufs=2) as nat,
            tc.tile_pool(name="lxt", bufs=1) as lxtp,
            tc.tile_pool(name="cvt", bufs=2) as cvtp,
            tc.tile_pool(name="wts", bufs=5) as wts,
            tc.tile_pool(name="gac", bufs=1) as gac,
            tc.tile_pool(name="fin", bufs=1) as fin,
            tc.tile_pool(name="pcv", bufs=2, space="PSUM") as pcv,
            tc.tile_pool(name="ptr", bufs=2, space="PSUM") as ptr,
            tc.tile_pool(name="pga", bufs=2, space="PSUM") as pga,
            tc.tile_pool(name="dram", bufs=1, space="DRAM") as dram,
        ):
            # ---- HAM warmup: ~4.5us of dense junk bf16 matmuls so the PE
            # clock-gate opens (K=8/8) before the real work. Output unread.
            warm_t = const.tile([128, 512], dt.bfloat16)
            nc.scalar.dma_start(warm_t, WarmI[:, :])
            warm_ps = ptr.tile([128, 512], dt.float32, tag="tr")
            for _ in range(10):
                nc.tensor.matmul(warm_ps, warm_t[:, 0:128], warm_t,
                                 start=True, stop=True)

            m_t = const.tile([N, N], dt.float32)
            nc.scalar.dma_start(m_t, Mm[:, :])
            i_t = const.tile([N, N], dt.float32)
            nc.scalar.dma_start(i_t, I24[:, :])

            def lap_transpose(src_t, name):
                """LT = (Mp @ src).T as [128, 32*24] sbuf tile (block k at
                cols k*24). 4 PE transposes share one psum tile + 1 copy."""
                lt = lxtp.tile([128, KB * N], dt.float32, name=name)
                for kc in range(KB // 4):
                    p = ptr.tile([128, 4 * N], dt.float32, tag="tr")
                    for j in range(4):
                        k = 4 * kc + j
                        nc.tensor.matmul(p[:, j * N:(j + 1) * N],
                                         src_t[:, k * 128:(k + 1) * 128],
                                         m_t, start=True, stop=True)
                    nc.vector.tensor_copy(
                        lt[:, kc * 4 * N:(kc + 1) * 4 * N], p)
                return lt

            lxt = lxtp.tile([128, KB * N], dt.float32, name="lxt_x")
            nc.scalar.dma_start(lxt, LXT[:, :])
            lht = lxtp.tile([128, KB * N], dt.float32, name="lht_h")
            nc.scalar.dma_start(lht, LHT[:, :])
            # independent tail/aux inputs: load early so they never queue
            # behind collective-waiting reads on the ACT HWDGE ring
            br_v = fin.tile([128, FV], dt.float32, name="br_v")
            nc.scalar.dma_start(br_v, _flat_view(Br))
            hsrc_v = fin.tile([128, FV], dt.float32, name="hsrc_v")
            nc.scalar.dma_start(hsrc_v, _flat_view(Hn))
            bz_s = fin.tile([N, CL], dt.float32, name="bz_s")
            nc.scalar.dma_start(bz_s, BzS[:, :])
            bh_s = fin.tile([N, CL], dt.float32, name="bh_s")
            nc.scalar.dma_start(bh_s, BhS[:, :])
            h_s = fin.tile([N, CL], dt.float32, name="h_s")
            nc.scalar.dma_start(h_s, HS[:, :])

            def conv_branch(lt, wdram, name):
                """CV.T = relu(LT.T @ W_local).T as [128, 4*24] (block j at
                cols j*24). Streams W (4096x512) in 2MiB chunks."""
                cv_ps = pcv.tile([N, CL], dt.float32, tag="cv")
                for kc in range(KB // 8):  # 4 chunks of 8 k-blocks (2 MiB)
                    wt = wts.tile([128, 8 * CL], dt.float32, tag="w")
                    nc.sync.dma_start(
                        wt.rearrange("p (k n) -> p k n", n=CL),
                        wdram.rearrange("(k p) n -> p k n", p=128)[
                            :, 8 * kc:8 * kc + 8, :],
                    )
                    for j in range(8):
                        k = 8 * kc + j
                        nc.tensor.matmul(
                            cv_ps,
                            lt[:, k * N:(k + 1) * N],
                            wt[:, j * CL:(j + 1) * CL],
                            start=(k == 0), stop=(k == KB - 1),
                        )
                cv = cvtp.tile([N, CL], dt.float32, tag="cv_sb", name=name,
                               bufs=3)
                relu_i = nc.scalar.activation(cv, cv_ps, AF.Relu)
                cvt = cvtp.tile([128, KBL * N], dt.float32, tag="cvt",
                                name=name + "_t", bufs=6)
                p = ptr.tile([128, KBL * N], dt.float32, tag="tr")
                for j in range(KBL):
                    nc.tensor.matmul(p[:, j * N:(j + 1) * N],
                                     cv[:, j * 128:(j + 1) * 128], i_t,
                                     start=True, stop=True)
                copy_i = nc.vector.tensor_copy(cvt, p)
                return cvt, relu_i, copy_i

            def gate_partial(cvt_x, u_x, cvt_h, u_h, bounce_in, a_major):
                """G_partial [24, 4096] = cvt_x.T @ u_x + cvt_h.T @ u_h.
                a_major=True writes the bounce as [8, 24, 512] (a-slice
                major) so ReduceScatter chunks are column slices."""
                g_sb = gac.tile([N, C], dt.float32, tag="g")
                uts = {}
                for a in range(NSL):
                    gp = pga.tile([N, 512], dt.float32, tag="ga")
                    for bi, (cvt, u) in enumerate(((cvt_x, u_x),
                                                   (cvt_h, u_h))):
                        if a % 2 == 0:
                            ut = wts.tile([128, KBL * 1024], dt.float32,
                                          tag="w", name=f"ut{bi}")
                            nc.sync.dma_start(
                                ut.rearrange("p (k n) -> p k n", n=1024),
                                u.rearrange("(k p) n -> p k n", p=128)[
                                    :, :, a * 512:(a + 2) * 512],
                            )
                            uts[bi] = ut
                        ut = uts[bi]
                        off = (a % 2) * 512
                        for j in range(KBL):
                            nc.tensor.matmul(
                                gp,
                                cvt[:, j * N:(j + 1) * N],
                                ut[:, j * 1024 + off:j * 1024 + off + 512],
                                start=(bi == 0 and j == 0),
                                stop=(bi == 1 and j == KBL - 1),
                            )
                    nc.vector.tensor_copy(g_sb[:, a * 512:(a + 1) * 512], gp)
                if a_major:
                    nc.scalar.dma_start(
                        bounce_in.rearrange("(a n) c -> n a c", n=N),
                        g_sb.rearrange("n (a c) -> n a c", c=512))
                else:
                    nc.scalar.dma_start(bounce_in[:, :], g_sb)

            r_in = dram.tile([N, C], dt.float32, name="cc_in_r")
            r_out = dram.tile([N, C], dt.float32, name="cc_out_r",
                              addr_space="Shared")
            z_in = dram.tile([NSL * N, 512], dt.float32, name="cc_in_z")
            z_out = dram.tile([N, CL], dt.float32, name="cc_out_z")
            h_in = dram.tile([NSL * N, 512], dt.float32, name="cc_in_h")
            h_out = dram.tile([N, CL], dt.float32, name="cc_out_h")

            # ---------------- r gate (first: R is needed downstream) ------
            cvt_xr, relu_xr, copy_xr = conv_branch(lxt, convW["xr"], "cv_xr")
            cvt_hr, relu_hr, copy_hr = conv_branch(lht, convW["hr"], "cv_hr")
            gate_partial(cvt_xr, gateU["xr"], cvt_hr, gateU["hr"], r_in,
                         a_major=False)
            nc.gpsimd.collective_compute(
                "AllReduce", ALU.add, replica_groups=REPLICAS,
                ins=[r_in.opt()], outs=[r_out.opt()])

            # -------- xh conv while the AR is in flight -------------------
            cvt_xh, relu_xh, copy_xh = conv_branch(lxt, convW["xh"], "cv_xh")

            # -------- R -> H*R on DVE/ACT (overlaps the z-gate PE work) ---
            gr_v = fin.tile([128, FV], dt.float32, name="gr_v")
            nc.scalar.dma_start(gr_v, _flat_view(r_out))
            r_v = fin.tile([128, FV], dt.float32, name="r_v")
            radd_i = nc.vector.tensor_add(r_v, gr_v, br_v)
            sig_r = nc.scalar.activation(r_v, r_v, AF.Sigmoid)
            hr_v = fin.tile([128, FV], dt.float32, name="hr_v")
            nc.vector.tensor_mul(hr_v, hsrc_v, r_v)
            hr_b = dram.tile([N, C], dt.float32, name="hr_bounce")
            nc.scalar.dma_start(_flat_view(hr_b), hr_v)
            hr_t = nat.tile([N, C], dt.float32, tag="natsrc", name="hr_t")
            nc.scalar.dma_start(hr_t, hr_b[:, :])

            # ---------------- z gate (keeps the PE busy during AR_r) ------
            cvt_xz, relu_xz, copy_xz = conv_branch(lxt, convW["xz"], "cv_xz")
            cvt_hz, relu_hz, copy_hz = conv_branch(lht, convW["hz"], "cv_hz")
            from concourse.tile import add_dep_helper
            add_dep_helper(radd_i.ins, copy_xz.ins, sync=False,
                           reason="DVE conv copies before R-chain add")
            add_dep_helper(radd_i.ins, copy_hz.ins, sync=False,
                           reason="DVE conv copies before R-chain add")
            add_dep_helper(sig_r.ins, relu_xz.ins, sync=False,
                           reason="ACT relus before R sigmoid")
            add_dep_helper(sig_r.ins, relu_hz.ins, sync=False,
                           reason="ACT relus before R sigmoid")
            gate_partial(cvt_xz, gateU["xz"], cvt_hz, gateU["hz"], z_in,
                         a_major=True)
            nc.gpsimd.collective_compute(
                "ReduceScatter", ALU.add, replica_groups=REPLICAS,
                ins=[z_in.opt()], outs=[z_out.opt()])

            # ---------------- h gate --------------------------------------
            lrt = lap_transpose(hr_t, "lrt_hr")
            cvt_hh, relu_hh, copy_hh = conv_branch(lrt, convW["hh"], "cv_hh")
            gate_partial(cvt_xh, gateU["xh"], cvt_hh, gateU["hh"], h_in,
                         a_major=True)
            nc.gpsimd.collective_compute(
                "ReduceScatter", ALU.add, replica_groups=REPLICAS,
                ins=[h_in.opt()], outs=[h_out.opt()])

            # -------- final combine on this core's column slice -----------
            gz_s = fin.tile([N, CL], dt.float32, name="gz_s")
            nc.scalar.dma_start(gz_s, z_out[:, :])
            z_s = fin.tile([N, CL], dt.float32, name="z_s")
            zadd_i = nc.vector.tensor_add(z_s, gz_s, bz_s)
            sig_z = nc.scalar.activation(z_s, z_s, AF.Sigmoid)
            add_dep_helper(sig_z.ins, relu_hh.ins, sync=False,
                           reason="hh relu before Z sigmoid")
            add_dep_helper(zadd_i.ins, copy_hh.ins, sync=False,
                           reason="hh cvt copy before Z add")

            gh_s = fin.tile([N, CL], dt.float32, name="gh_s")
            nc.scalar.dma_start(gh_s, h_out[:, :])
            ht_s = fin.tile([N, CL], dt.float32, name="ht_s")
            nc.vector.tensor_add(ht_s, gh_s, bh_s)
            nc.scalar.activation(ht_s, ht_s, AF.Tanh)

            o_s = fin.tile([N, CL], dt.float32, name="o_s")
            # out = H + Z * (Ht - H)
            nc.vector.tensor_sub(o_s, ht_s, h_s)
            nc.vector.tensor_mul(o_s, o_s, z_s)
            nc.vector.tensor_add(o_s, o_s, h_s)
            nc.scalar.dma_start(out[:, :], o_s)

    nc.compile()
    return nc


def _host_prep(X, edge_index, edge_weight, H, lambda_max,
               Wc_xz, Wc_hz, Wc_xr, Wc_hr, Wc_xh, Wc_hh,
               w_x_z, w_q_z, w_x_r, w_q_r, w_x_h, w_q_h,
               b_z, b_r, b_h):
    """Build per-core input maps."""
    import ml_dtypes
    f32 = np.float32
    X = np.ascontiguousarray(np.asarray(X, f32))
    H = np.ascontiguousarray(np.asarray(H, f32))
    ei = np.asarray(edge_index)
    ew = np.asarray(edge_weight, np.float64)
    lam = float(np.asarray(lambda_max).reshape(-1)[0])

    src = ei[0].astype(np.int64)
    dst = ei[1].astype(np.int64)
    deg_s = np.bincount(src, weights=ew, minlength=N)[:N]
    deg_d = np.bincount(dst, weights=ew, minlength=N)[:N]
    dinv_s = np.where(deg_s > 0, 1.0 / np.sqrt(np.maximum(deg_s, 1e-12)), 0.0)
    dinv_d = np.where(deg_d > 0, 1.0 / np.sqrt(np.maximum(deg_d, 1e-12)), 0.0)
    norm = ew * dinv_s[src] * dinv_d[dst]
    A = np.zeros((N, N), np.float64)
    np.add.at(A, (dst, src), norm)
    Mp = (2.0 / lam - 1.0) * np.eye(N) - (2.0 / lam) * A   # LX = Mp @ T
    Mm = np.ascontiguousarray(Mp.T).astype(f32)            # device rhs [j, i]
    I = np.eye(N, dtype=f32)

    conv = {"xz": Wc_xz, "hz": Wc_hz, "xr": Wc_xr,
            "hr": Wc_hr, "xh": Wc_xh, "hh": Wc_hh}
    gate = {"xz": w_x_z, "hz": w_q_z, "xr": w_x_r,
            "hr": w_q_r, "xh": w_x_h, "hh": w_q_h}
    conv = {k: np.asarray(v, f32) for k, v in conv.items()}
    gate = {k: np.asarray(v, f32) for k, v in gate.items()}
    b_z = np.asarray(b_z, f32)
    b_h = np.asarray(b_h, f32)

    def tile_layout(lt):
        # LX.T [C, N] -> [128, 32*24] with block k at cols k*24
        return np.ascontiguousarray(
            lt.reshape(KB, 128, N).transpose(1, 0, 2).reshape(128, KB * N))

    LX = (Mp @ X.astype(np.float64))
    LH = (Mp @ H.astype(np.float64))
    common = {
        "Hn": H, "Mm": Mm, "I24": I,
        "WarmI": np.zeros((128, 512), ml_dtypes.bfloat16),
        "LXT": tile_layout(LX.T.astype(f32)),
        "LHT": tile_layout(LH.T.astype(f32)),
        "Br": np.ascontiguousarray(np.asarray(b_r, f32)),
    }
    in_maps = []
    for c in range(NCORES):
        sl = slice(c * CL, (c + 1) * CL)
        m = dict(common)
        for g in ("xz", "hz", "xr", "hr", "xh", "hh"):
            m[f"W{g}"] = np.ascontiguousarray(conv[g][:, sl])
            m[f"U{g}"] = np.ascontiguousarray(gate[g][:, sl].T)
        m["BzS"] = np.ascontiguousarray(b_z[:, sl])
        m["BhS"] = np.ascontiguousarray(b_h[:, sl])
        m["HS"] = np.ascontiguousarray(H[:, sl])
        in_maps.append(m)
    return in_maps


def kernel(**inputs):
    global LAST_RESULT
    if "nc" not in _CACHE:
        _CACHE["nc"] = _build()
    nc = _CACHE["nc"]
    in_maps = _host_prep(**inputs)
    res = run_bass_kernel_spmd(nc, in_maps, list(range(NCORES)), **RUN_KWARGS)
    LAST_RESULT = res
    return np.concatenate(
        [np.asarray(res.results[c]["out"], np.float32)
         for c in range(NCORES)], axis=1)



# revision 3
# speedup vs baseline: 2.2540x; 2.2540x over previous
"""Trainium2 Bass kernel for RecGRU_W_up (gnn_message_passing).

Computes, for N=24 nodes / C=4096 channels / 8 NeuronCores:
    Z = sigmoid(conv(X,Wc_xz) @ w_x_z.T + conv(H,Wc_hz) @ w_q_z.T + b_z)
    R = sigmoid(conv(X,Wc_xr) @ w_x_r.T + conv(H,Wc_hr) @ w_q_r.T + b_r)
    Ht = tanh(conv(X,Wc_xh) @ w_x_h.T + conv(H*R,Wc_hh) @ w_q_h.T + b_h)
    out = Z*Ht + (1-Z)*H
where conv(T, W) = relu(LX @ W), LX = (2/lam)(T - A_norm T) - T.

Sharding / numerics:
  - All 12 C-by-C weight matrices are column-sharded (output dim) and
    stored as fp8 e3m4 scaled by 128 (descaled by 1/128^2 inside the
    conv relu); activations are bf16; PSUM/elementwise fp32.
  - Each core produces conv outputs for its 512-column slice, locally
    transposes them, and small bf16 AllGathers replicate the transposed
    conv outputs to every core. Gate GEMMs then run with the full
    C=4096 contraction against the local 512-wide gate-weight slice, so
    gate outputs are already the local column slice -- no AllReduce /
    ReduceScatter anywhere.
  - Graph propagation is a [24,24] matrix Mp folded on the host; L(X).T
    and L(H).T are host-precomputed; L(H*R).T is built per-slice on the
    PE and AllGathered.
  - Matmuls are packed 2-3 per issue into distinct PE column groups
    (tile_position) so branches run concurrently on the 128x128 array.
"""

import numpy as np

import concourse.bass as bass
import concourse.mybir as mybir
import concourse.tile as tile
from concourse import bacc
from concourse.bass_utils import run_bass_kernel_spmd

dt = mybir.dt
AF = mybir.ActivationFunctionType
ALU = mybir.AluOpType

N = 24
C = 4096
NCORES = 8
CL = C // NCORES          # 512 channels per core
KB = C // 128             # 32 k-blocks over full C
KBL = CL // 128           # 4 k-blocks over the local slice
HKB = KB // 2             # 16 k-blocks per half-matrix DMA chunk
SCALE = 128.0             # fp8 weight scale
S2I = 1.0 / (SCALE * SCALE)

REPLICAS = [list(range(NCORES))]

_CACHE = {}
RUN_KWARGS = {}
LAST_RESULT = None


def _build():
    nc = bacc.Bacc("TRN2", target_bir_lowering=False, debug=False,
                   num_devices=NCORES)

    # ---- I/O ----
    LXT = nc.dram_tensor("LXT", [128, KB * N], dt.bfloat16,
                         kind="ExternalInput")
    LHT = nc.dram_tensor("LHT", [128, KB * N], dt.bfloat16,
                         kind="ExternalInput")
    Mm = nc.dram_tensor("Mm", [N, N], dt.bfloat16, kind="ExternalInput")
    I24 = nc.dram_tensor("I24", [N, N], dt.bfloat16, kind="ExternalInput")
    WarmI = nc.dram_tensor("WarmI", [128, 512], dt.bfloat16,
                           kind="ExternalInput")
    convW = {}
    gateU = {}
    for g in ("xz", "hz", "xr", "hr", "xh", "hh"):
        convW[g] = nc.dram_tensor(f"W{g}", [C, CL], dt.float8e3,
                                  kind="ExternalInput")
        gateU[g] = nc.dram_tensor(f"U{g}", [C, CL], dt.float8e3,
                                  kind="ExternalInput")
    BrS = nc.dram_tensor("BrS", [N, CL], dt.float32, kind="ExternalInput")
    BzS = nc.dram_tensor("BzS", [N, CL], dt.float32, kind="ExternalInput")
    BhS = nc.dram_tensor("BhS", [N, CL], dt.float32, kind="ExternalInput")
    HS = nc.dram_tensor("HS", [N, CL], dt.float32, kind="ExternalInput")
    out = nc.dram_tensor("out", [N, CL], dt.float32, kind="ExternalOutput")

    with tile.TileContext(nc) as tc:
        with (
            tc.tile_pool(name="const", bufs=1) as const,
            tc.tile_pool(name="wts", bufs=8) as wts,
            tc.tile_pool(name="cvt", bufs=1) as cvtp,
            tc.tile_pool(name="ag", bufs=1) as agp,
            tc.tile_pool(name="fin", bufs=1) as fin,
            tc.tile_pool(name="pcv", bufs=2, space="PSUM") as pcv,
            tc.tile_pool(name="ptr", bufs=2, space="PSUM") as ptr,
            tc.tile_pool(name="pga", bufs=2, space="PSUM") as pga,
            tc.tile_pool(name="dram", bufs=1, space="DRAM") as dram,
        ):
            # ---- HAM warmup: junk bf16 matmuls so the PE clock-gate
            # opens before the real work. Output unread.
            warm_t = const.tile([128, 512], dt.bfloat16)
            nc.scalar.dma_start(warm_t, WarmI[:, :])
            warm_ps = ptr.tile([128, 512], dt.float32, tag="warm")
            for _ in range(10):
                nc.tensor.matmul(warm_ps, warm_t[:, 0:128], warm_t,
                                 start=True, stop=True)

            m_t = const.tile([N, N], dt.bfloat16)
            nc.scalar.dma_start(m_t, Mm[:, :])
            i_t = const.tile([N, N], dt.bfloat16)
            nc.scalar.dma_start(i_t, I24[:, :])
            lxt = const.tile([128, KB * N], dt.bfloat16, name="lxt")
            nc.scalar.dma_start(lxt, LXT[:, :])
            lht = const.tile([128, KB * N], dt.bfloat16, name="lht")
            nc.scalar.dma_start(lht, LHT[:, :])
            br_s = const.tile([N, CL], dt.float32, name="br_s")
            nc.scalar.dma_start(br_s, BrS[:, :])
            bz_s = const.tile([N, CL], dt.float32, name="bz_s")
            nc.scalar.dma_start(bz_s, BzS[:, :])
            bh_s = const.tile([N, CL], dt.float32, name="bh_s")
            nc.scalar.dma_start(bh_s, BhS[:, :])
            h_s = const.tile([N, CL], dt.float32, name="h_s")
            nc.scalar.dma_start(h_s, HS[:, :])

            def wchunk(wdram, half, name):
                """One 1MiB half-matrix stream tile [128, HKB*CL] fp8."""
                wt = wts.tile([128, HKB * CL], dt.float8e3, tag="w",
                              name=name)
                nc.sync.dma_start(
                    wt.rearrange("p (k n) -> p k n", n=CL),
                    wdram.rearrange("(k p) n -> p k n", p=128)[
                        :, HKB * half:HKB * (half + 1), :],
                )
                return wt

            def conv_pack(branches, name):
                """Packed conv GEMMs: branches = [(lhsT_fn, wdram), ...]
                (<=3). Each branch b accumulates into psum partitions
                32b..32b+23 over all KB k-blocks. Returns list of
                transposed+relu'd cvt tiles [128, KBL*N] bf16 plus the
                instruction handles of the relu and the psum->sbuf copy."""
                nb = len(branches)
                pc = pcv.tile([128, CL], dt.float32, tag="cv")
                for half in range(2):
                    wtiles = [wchunk(w, half, f"w_{name}_{bi}_{half}")
                              for bi, (_, w) in enumerate(branches)]
                    for j in range(HKB):
                        k = HKB * half + j
                        for bi, (lf, _) in enumerate(branches):
                            nc.tensor.matmul(
                                pc[32 * bi:32 * bi + N, :],
                                lf(k),
                                wtiles[bi][:, j * CL:(j + 1) * CL],
                                start=(k == 0), stop=(k == KB - 1),
                                tile_position=(0, 32 * bi),
                            )
                cvts = []
                relus = []
                pt = ptr.tile([128, nb * KBL * N], dt.float32, tag="tr")
                cvt = cvtp.tile([128, nb * KBL * N], dt.bfloat16,
                                name=f"cvt_{name}", bufs=4)
                for bi in range(nb):
                    cv = cvtp.tile([N, CL], dt.bfloat16,
                                   name=f"cv_{name}_{bi}", bufs=4)
                    relus.append(nc.scalar.activation(
                        cv, pc[32 * bi:32 * bi + N, :], AF.Relu, scale=S2I))
                    for kk in range(KBL):
                        nc.tensor.matmul(
                            pt[:, (bi * KBL + kk) * N:(bi * KBL + kk + 1) * N],
                            cv[:, kk * 128:(kk + 1) * 128], i_t,
                            start=True, stop=True)
                copy_i = nc.vector.tensor_copy(cvt, pt)
                return cvt, relus, copy_i

            def allgather(src_sb, width, name):
                """AllGather a [128, width] bf16 tile; returns sbuf tile
                [128, NCORES*width] with rank r's block at cols r*width."""
                cin = dram.tile([128, width], dt.bfloat16,
                                name=f"agin_{name}")
                cout = dram.tile([NCORES * 128, width], dt.bfloat16,
                                 name=f"agout_{name}", addr_space="Shared")
                nc.scalar.dma_start(cin[:, :], src_sb)
                nc.gpsimd.collective_compute(
                    "AllGather", ALU.bypass, replica_groups=REPLICAS,
                    ins=[cin.opt()], outs=[cout.opt()])
                ag = agp.tile([128, NCORES * width], dt.bfloat16,
                              name=f"ag_{name}")
                nc.scalar.dma_start(
                    ag.rearrange("p (r f) -> p r f", r=NCORES),
                    cout.rearrange("(r p) f -> p r f", p=128))
                return ag

            def gate_pack(ag_x, xoff, xstride, ag_h, hoff, hstride,
                          udx, udh, name):
                """Gate GEMM over full C: branch x in col group 0, branch h
                in col group 1. lhsT k-block k=(r,kk) of branch x lives at
                ag_x[:, r*xstride + xoff + kk*N]. Returns psum tile."""
                pg = pga.tile([128, CL], dt.float32, tag="ga")
                for half in range(2):
                    ux = wchunk(udx, half, f"u_{name}_x_{half}")
                    uh = wchunk(udh, half, f"u_{name}_h_{half}")
                    for j in range(HKB):
                        k = HKB * half + j
                        r, kk = divmod(k, KBL)
                        nc.tensor.matmul(
                            pg[0:N, :],
                            ag_x[:, r * xstride + xoff + kk * N:
                                 r * xstride + xoff + (kk + 1) * N],
                            ux[:, j * CL:(j + 1) * CL],
                            start=(k == 0), stop=(k == KB - 1),
                            tile_position=(0, 0),
                        )
                        nc.tensor.matmul(
                            pg[32:32 + N, :],
                            ag_h[:, r * hstride + hoff + kk * N:
                                 r * hstride + hoff + (kk + 1) * N],
                            uh[:, j * CL:(j + 1) * CL],
                            start=(k == 0), stop=(k == KB - 1),
                            tile_position=(0, 32),
                        )
                return pg

            def gate_finish(pg, bias, func, name):
                """sigmoid/tanh(G_x + G_h + bias) -> [24, CL] fp32."""
                t1 = fin.tile([N, CL], dt.float32, name=f"g1_{name}")
                nc.scalar.activation(t1, pg[32:32 + N, :], AF.Copy)
                t2 = fin.tile([N, CL], dt.float32, name=f"g2_{name}")
                nc.vector.tensor_add(t2, pg[0:N, :], t1)
                t3 = fin.tile([N, CL], dt.float32, name=f"g3_{name}")
                nc.vector.tensor_add(t3, t2, bias)
                g = fin.tile([N, CL], dt.float32, name=f"g_{name}")
                nc.scalar.activation(g, t3, func)
                return g

            # ---------------- r chain (critical path) --------------------
            cvt_r, _, _ = conv_pack(
                [(lambda k: lxt[:, k * N:(k + 1) * N], convW["xr"]),
                 (lambda k: lht[:, k * N:(k + 1) * N], convW["hr"])], "r")
            ag_r = allgather(cvt_r, 2 * KBL * N, "r")
            pg_r = gate_pack(ag_r, 0, 2 * KBL * N, ag_r, KBL * N, 2 * KBL * N,
                             gateU["xr"], gateU["hr"], "r")
            r_s = gate_finish(pg_r, br_s, AF.Sigmoid, "r")

            # H*R -> local L(H*R).T blocks -> AllGather
            hr_s = fin.tile([N, CL], dt.bfloat16, name="hr_s")
            nc.vector.tensor_mul(hr_s, h_s, r_s)
            pl = ptr.tile([128, KBL * N], dt.float32, tag="tr")
            for kk in range(KBL):
                nc.tensor.matmul(pl[:, kk * N:(kk + 1) * N],
                                 hr_s[:, kk * 128:(kk + 1) * 128], m_t,
                                 start=True, stop=True)
            lrt_l = cvtp.tile([128, KBL * N], dt.bfloat16, name="lrt_l")
            nc.vector.tensor_copy(lrt_l, pl)
            ag_l = allgather(lrt_l, KBL * N, "lrt")

            # ---------------- z + xh convs (fill the AG_r shadow) ---------
            cvt_zx, _, _ = conv_pack(
                [(lambda k: lxt[:, k * N:(k + 1) * N], convW["xz"]),
                 (lambda k: lht[:, k * N:(k + 1) * N], convW["hz"]),
                 (lambda k: lxt[:, k * N:(k + 1) * N], convW["xh"])], "zx")
            ag_z = allgather(cvt_zx, 3 * KBL * N, "z")

            # ---------------- hh conv (needs AG_l) ------------------------
            cvt_hh, _, _ = conv_pack(
                [(lambda k: ag_l[:, (k // KBL) * (KBL * N)
                                 + (k % KBL) * N:
                                 (k // KBL) * (KBL * N)
                                 + (k % KBL + 1) * N], convW["hh"])], "hh")
            ag_hh = allgather(cvt_hh, KBL * N, "hh")

            # ---------------- gate h ------------------------------------
            pg_h = gate_pack(ag_z, 2 * KBL * N, 3 * KBL * N,
                             ag_hh, 0, KBL * N,
                             gateU["xh"], gateU["hh"], "h")
            ht_s = gate_finish(pg_h, bh_s, AF.Tanh, "h")

            # ---------------- gate z ------------------------------------
            pg_z = gate_pack(ag_z, 0, 3 * KBL * N, ag_z, KBL * N, 3 * KBL * N,
                             gateU["xz"], gateU["hz"], "z")
            z_s = gate_finish(pg_z, bz_s, AF.Sigmoid, "z")

            # -------- final combine: out = H + Z * (Ht - H) ---------------
            o_s = fin.tile([N, CL], dt.float32, name="o_s")
            nc.vector.tensor_sub(o_s, ht_s, h_s)
            nc.vector.tensor_mul(o_s, o_s, z_s)
            nc.vector.tensor_add(o_s, o_s, h_s)
            nc.scalar.dma_start(out[:, :], o_s)

    nc.compile()
    return nc


def _host_prep(X, edge_index, edge_weight, H, lambda_max,
               Wc_xz, Wc_hz, Wc_xr, Wc_hr, Wc_xh, Wc_hh,
               w_x_z, w_q_z, w_x_r, w_q_r, w_x_h, w_q_h,
               b_z, b_r, b_h):
    """Build per-core input maps."""
    import ml_dtypes
    f32 = np.float32
    bf16 = ml_dtypes.bfloat16
    f8 = ml_dtypes.float8_e3m4
    X = np.ascontiguousarray(np.asarray(X, f32))
    H = np.ascontiguousarray(np.asarray(H, f32))
    ei = np.asarray(edge_index)
    ew = np.asarray(edge_weight, np.float64)
    lam = float(np.asarray(lambda_max).reshape(-1)[0])

    src = ei[0].astype(np.int64)
    dst = ei[1].astype(np.int64)
    deg_s = np.bincount(src, weights=ew, minlength=N)[:N]
    deg_d = np.bincount(dst, weights=ew, minlength=N)[:N]
    dinv_s = np.where(deg_s > 0, 1.0 / np.sqrt(np.maximum(deg_s, 1e-12)), 0.0)
    dinv_d = np.where(deg_d > 0, 1.0 / np.sqrt(np.maximum(deg_d, 1e-12)), 0.0)
    norm = ew * dinv_s[src] * dinv_d[dst]
    A = np.zeros((N, N), np.float64)
    np.add.at(A, (dst, src), norm)
    Mp = (2.0 / lam - 1.0) * np.eye(N) - (2.0 / lam) * A   # LX = Mp @ T
    Mm = np.ascontiguousarray(Mp.T).astype(bf16)           # device rhs [j, i]
    I = np.eye(N, dtype=bf16)

    conv = {"xz": Wc_xz, "hz": Wc_hz, "xr": Wc_xr,
            "hr": Wc_hr, "xh": Wc_xh, "hh": Wc_hh}
    gate = {"xz": w_x_z, "hz": w_q_z, "xr": w_x_r,
            "hr": w_q_r, "xh": w_x_h, "hh": w_q_h}
    conv = {k: np.asarray(v, f32) for k, v in conv.items()}
    gate = {k: np.asarray(v, f32) for k, v in gate.items()}
    b_r = np.asarray(b_r, f32)
    b_z = np.asarray(b_z, f32)
    b_h = np.asarray(b_h, f32)

    def q8(w):
        return np.clip(np.asarray(w, np.float64) * SCALE,
                       -15.5, 15.5).astype(f8)

    def tile_layout(lt):
        # LX.T [C, N] -> [128, 32*24] with block k at cols k*24
        return np.ascontiguousarray(
            lt.reshape(KB, 128, N).transpose(1, 0, 2).reshape(128, KB * N))

    LX = (Mp @ X.astype(np.float64))
    LH = (Mp @ H.astype(np.float64))
    common = {
        "Mm": Mm, "I24": I,
        "WarmI": np.zeros((128, 512), bf16),
        "LXT": tile_layout(LX.T).astype(bf16),
        "LHT": tile_layout(LH.T).astype(bf16),
    }
    in_maps = []
    for c in range(NCORES):
        sl = slice(c * CL, (c + 1) * CL)
        m = dict(common)
        for g in ("xz", "hz", "xr", "hr", "xh", "hh"):
            m[f"W{g}"] = np.ascontiguousarray(q8(conv[g][:, sl]))
            m[f"U{g}"] = np.ascontiguousarray(q8(gate[g][sl, :].T))
        m["BrS"] = np.ascontiguousarray(b_r[:, sl])
        m["BzS"] = np.ascontiguousarray(b_z[:, sl])
        m["BhS"] = np.ascontiguousarray(b_h[:, sl])
        m["HS"] = np.ascontiguousarray(H[:, sl])
        in_maps.append(m)
    return in_maps


def kernel(**inputs):
    global LAST_RESULT
    if "nc" not in _CACHE:
        _CACHE["nc"] = _build()
    nc = _CACHE["nc"]
    in_maps = _host_prep(**inputs)
    res = run_bass_kernel_spmd(nc, in_maps, list(range(NCORES)), **RUN_KWARGS)
    LAST_RESULT = res
    return np.concatenate(
        [np.asarray(res.results[c]["out"], np.float32)
         for c in range(NCORES)], axis=1)


# revision 11
# speedup vs baseline: 2.4262x; 1.0764x over previous
"""Trainium2 Bass kernel for RecGRU_W_up (gnn_message_passing).

Computes, for N=24 nodes / C=4096 channels / 8 NeuronCores:
    Z = sigmoid(conv(X,Wc_xz) @ w_x_z.T + conv(H,Wc_hz) @ w_q_z.T + b_z)
    R = sigmoid(conv(X,Wc_xr) @ w_x_r.T + conv(H,Wc_hr) @ w_q_r.T + b_r)
    Ht = tanh(conv(X,Wc_xh) @ w_x_h.T + conv(H*R,Wc_hh) @ w_q_h.T + b_h)
    out = Z*Ht + (1-Z)*H
where conv(T, W) = relu(LX @ W), LX = (2/lam)(T - A_norm T) - T.

Sharding / numerics:
  - All 12 C-by-C weight matrices are column-sharded (output dim) and
    stored as fp8 e3m4 scaled by 128 (descaled by 1/128^2 inside the
    conv relu); activations are bf16; PSUM/elementwise fp32.
  - Each core produces conv outputs for its 512-column slice, locally
    transposes them, and small bf16 AllGathers replicate the transposed
    conv outputs to every core. Gate GEMMs then run with the full
    C=4096 contraction against the local 512-wide gate-weight slice, so
    gate outputs are already the local column slice -- no AllReduce /
    ReduceScatter anywhere.
  - Graph propagation is a [24,24] matrix Mp folded on the host; L(X).T
    and L(H).T are host-precomputed; L(H*R).T is built per-slice on the
    PE and AllGathered.
  - Matmuls are packed 2-3 per issue into distinct PE column groups
    (tile_position) so branches run concurrently on the 128x128 array.
"""

import numpy as np

import concourse.bass as bass
import concourse.mybir as mybir
import concourse.tile as tile
from concourse import bacc
from concourse.bass_utils import run_bass_kernel_spmd

dt = mybir.dt
AF = mybir.ActivationFunctionType
ALU = mybir.AluOpType

N = 24
C = 4096
NCORES = 8
CL = C // NCORES          # 512 channels per core
KB = C // 128             # 32 k-blocks over full C
KBL = CL // 128           # 4 k-blocks over the local slice
HKB = KB // 2             # 16 k-blocks per half-matrix DMA chunk
SCALE = 128.0             # fp8 weight scale
S2I = 1.0 / (SCALE * SCALE)

REPLICAS = [list(range(NCORES))]

_CACHE = {}
RUN_KWARGS = {}
LAST_RESULT = None


def _build():
    nc = bacc.Bacc("TRN2", target_bir_lowering=False, debug=False,
                   num_devices=NCORES)

    # ---- I/O ----
    LXT = nc.dram_tensor("LXT", [128, KB * N], dt.bfloat16,
                         kind="ExternalInput")
    LHT = nc.dram_tensor("LHT", [128, KB * N], dt.bfloat16,
                         kind="ExternalInput")
    Mm = nc.dram_tensor("Mm", [N, N], dt.bfloat16, kind="ExternalInput")
    I24 = nc.dram_tensor("I24", [N, N], dt.bfloat16, kind="ExternalInput")
    WarmI = nc.dram_tensor("WarmI", [128, 512], dt.bfloat16,
                           kind="ExternalInput")
    convW = {}
    gateU = {}
    # weights arrive host-pre-tiled: [128, KB*CL] with k-block k at
    # cols k*CL (contiguous per partition -> line-rate DMA descriptors)
    for g in ("xz", "hz", "xr", "hr", "xh", "hh"):
        convW[g] = nc.dram_tensor(f"W{g}", [128, KB * CL], dt.float8e3,
                                  kind="ExternalInput")
        gateU[g] = nc.dram_tensor(f"U{g}", [128, KB * CL], dt.float8e3,
                                  kind="ExternalInput")
    BrS = nc.dram_tensor("BrS", [N, CL], dt.float32, kind="ExternalInput")
    BzS = nc.dram_tensor("BzS", [N, CL], dt.float32, kind="ExternalInput")
    BhS = nc.dram_tensor("BhS", [N, CL], dt.float32, kind="ExternalInput")
    HS = nc.dram_tensor("HS", [N, CL], dt.float32, kind="ExternalInput")
    out = nc.dram_tensor("out", [N, CL], dt.float32, kind="ExternalOutput")

    with tile.TileContext(nc) as tc:
        with (
            tc.tile_pool(name="const", bufs=1) as const,
            tc.tile_pool(name="wts", bufs=12) as wts,
            tc.tile_pool(name="cvt", bufs=1) as cvtp,
            tc.tile_pool(name="ag", bufs=1) as agp,
            tc.tile_pool(name="fin", bufs=1) as fin,
            tc.tile_pool(name="pcv", bufs=2, space="PSUM") as pcv,
            tc.tile_pool(name="ptr", bufs=2, space="PSUM") as ptr,
            tc.tile_pool(name="pga", bufs=2, space="PSUM") as pga,
            tc.tile_pool(name="dram", bufs=1, space="DRAM") as dram,
        ):
            # ---- HAM warmup: junk bf16 matmuls so the PE clock-gate
            # opens before the real work. Output unread.
            warm_t = const.tile([128, 512], dt.bfloat16)
            nc.scalar.dma_start(warm_t, WarmI[:, :])
            warm_ps = ptr.tile([128, 512], dt.float32, tag="warm")
            for _ in range(10):
                nc.tensor.matmul(warm_ps, warm_t[:, 0:128], warm_t,
                                 start=True, stop=True)

            # Dummy tiny AllGather issued first: absorbs the ncfw channel
            # bootstrap cost (~40us) so the real collectives run at the
            # steady-state ~10us latency. Output intentionally unread.
            dum_in = dram.tile([128, 32], dt.bfloat16, name="dummy_in")
            dum_out = dram.tile([NCORES * 128, 32], dt.bfloat16,
                                name="dummy_out", addr_space="Shared")
            nc.scalar.dma_start(dum_in[:, :], warm_t[:, 0:32])
            nc.gpsimd.collective_compute(
                "AllGather", ALU.bypass, replica_groups=REPLICAS,
                ins=[dum_in.opt()], outs=[dum_out.opt()])

            m_t = const.tile([N, N], dt.bfloat16)
            nc.scalar.dma_start(m_t, Mm[:, :])
            i_t = const.tile([N, N], dt.bfloat16)
            nc.scalar.dma_start(i_t, I24[:, :])
            lxt = const.tile([128, KB * N], dt.bfloat16, name="lxt")
            nc.scalar.dma_start(lxt, LXT[:, :])
            lht = const.tile([128, KB * N], dt.bfloat16, name="lht")
            nc.scalar.dma_start(lht, LHT[:, :])
            br_s = const.tile([N, CL], dt.float32, name="br_s")
            nc.scalar.dma_start(br_s, BrS[:, :])
            bz_s = const.tile([N, CL], dt.float32, name="bz_s")
            nc.scalar.dma_start(bz_s, BzS[:, :])
            bh_s = const.tile([N, CL], dt.float32, name="bh_s")
            nc.scalar.dma_start(bh_s, BhS[:, :])
            h_s = const.tile([N, CL], dt.float32, name="h_s")
            nc.scalar.dma_start(h_s, HS[:, :])

            def wchunk(wdram, half, name):
                """One 1MiB half-matrix stream tile [128, HKB*CL] fp8."""
                wt = wts.tile([128, HKB * CL], dt.float8e3, tag="w",
                              name=name)
                nc.sync.dma_start(
                    wt, wdram[:, HKB * CL * half:HKB * CL * (half + 1)])
                return wt

            def conv_pack(branches, name):
                """Packed conv GEMMs: branches = [(lhsT_fn, wdram), ...]
                (<=3). Each branch b accumulates into psum partitions
                32b..32b+23 over all KB k-blocks. Returns list of
                transposed+relu'd cvt tiles [128, KBL*N] bf16 plus the
                instruction handles of the relu and the psum->sbuf copy."""
                nb = len(branches)
                pc = pcv.tile([128, CL], dt.float32, tag="cv")
                for half in range(2):
                    wtiles = [wchunk(w, half, f"w_{name}_{bi}_{half}")
                              for bi, (_, w) in enumerate(branches)]
                    for j in range(HKB):
                        k = HKB * half + j
                        for bi, (lf, _) in enumerate(branches):
                            nc.tensor.matmul(
                                pc[32 * bi:32 * bi + N, :],
                                lf(k),
                                wtiles[bi][:, j * CL:(j + 1) * CL],
                                start=(k == 0), stop=(k == KB - 1),
                                tile_position=(0, 32 * bi),
                            )
                cvts = []
                relus = []
                pt = ptr.tile([128, nb * KBL * N], dt.float32, tag="tr")
                cvt = cvtp.tile([128, nb * KBL * N], dt.bfloat16,
                                name=f"cvt_{name}", bufs=4)
                for bi in range(nb):
                    cv = cvtp.tile([N, CL], dt.bfloat16,
                                   name=f"cv_{name}_{bi}", bufs=4)
                    relus.append(nc.scalar.activation(
                        cv, pc[32 * bi:32 * bi + N, :], AF.Relu, scale=S2I))
                    for kk in range(KBL):
                        nc.tensor.matmul(
                            pt[:, (bi * KBL + kk) * N:(bi * KBL + kk + 1) * N],
                            cv[:, kk * 128:(kk + 1) * 128], i_t,
                            start=True, stop=True)
                copy_i = nc.vector.tensor_copy(cvt, pt)
                return cvt, relus, copy_i

            def ag_start(src_sb, width, name):
                """Bounce a [128, width] bf16 tile to DRAM and launch its
                AllGather; returns the shared output handle."""
                cin = dram.tile([128, width], dt.bfloat16,
                                name=f"agin_{name}")
                cout = dram.tile([NCORES * 128, width], dt.bfloat16,
                                 name=f"agout_{name}", addr_space="Shared")
                nc.scalar.dma_start(cin[:, :], src_sb)
                nc.gpsimd.collective_compute(
                    "AllGather", ALU.bypass, replica_groups=REPLICAS,
                    ins=[cin.opt()], outs=[cout.opt()])
                return cout, width, name

            def ag_read(handle):
                """Read an AllGather result into sbuf [128, NCORES*width],
                rank r's block at cols r*width."""
                cout, width, name = handle
                ag = agp.tile([128, NCORES * width], dt.bfloat16,
                              name=f"ag_{name}")
                nc.scalar.dma_start(
                    ag.rearrange("p (r f) -> p r f", r=NCORES),
                    cout.rearrange("(r p) f -> p r f", p=128))
                return ag

            def gate_pack(ag_x, xoff, xstride, ag_h, hoff, hstride,
                          udx, udh, name):
                """Gate GEMM over full C: branch x in col group 0, branch h
                in col group 1. lhsT k-block k=(r,kk) of branch x lives at
                ag_x[:, r*xstride + xoff + kk*N]. Returns psum tile."""
                pg = pga.tile([128, CL], dt.float32, tag="ga")
                for half in range(2):
                    ux = wchunk(udx, half, f"u_{name}_x_{half}")
                    uh = wchunk(udh, half, f"u_{name}_h_{half}")
                    for j in range(HKB):
                        k = HKB * half + j
                        r, kk = divmod(k, KBL)
                        nc.tensor.matmul(
                            pg[0:N, :],
                            ag_x[:, r * xstride + xoff + kk * N:
                                 r * xstride + xoff + (kk + 1) * N],
                            ux[:, j * CL:(j + 1) * CL],
                            start=(k == 0), stop=(k == KB - 1),
                            tile_position=(0, 0),
                        )
                        nc.tensor.matmul(
                            pg[32:32 + N, :],
                            ag_h[:, r * hstride + hoff + kk * N:
                                 r * hstride + hoff + (kk + 1) * N],
                            uh[:, j * CL:(j + 1) * CL],
                            start=(k == 0), stop=(k == KB - 1),
                            tile_position=(0, 32),
                        )
                return pg

            def gate_finish(pg, bias, func, name):
                """sigmoid/tanh(G_x + G_h + bias) -> [24, CL] fp32."""
                t1 = fin.tile([N, CL], dt.float32, name=f"g1_{name}")
                nc.scalar.activation(t1, pg[32:32 + N, :], AF.Copy)
                t2 = fin.tile([N, CL], dt.float32, name=f"g2_{name}")
                nc.vector.tensor_add(t2, pg[0:N, :], t1)
                t3 = fin.tile([N, CL], dt.float32, name=f"g3_{name}")
                nc.vector.tensor_add(t3, t2, bias)
                g = fin.tile([N, CL], dt.float32, name=f"g_{name}")
                nc.scalar.activation(g, t3, func)
                return g

            # ---------------- r chain (critical path) --------------------
            cvt_r, _, _ = conv_pack(
                [(lambda k: lxt[:, k * N:(k + 1) * N], convW["xr"]),
                 (lambda k: lht[:, k * N:(k + 1) * N], convW["hr"])], "r")
            agh_r = ag_start(cvt_r, 2 * KBL * N, "r")

            # ---------------- z + xh convs (fill the AG_r shadow) ---------
            cvt_zx, _, _ = conv_pack(
                [(lambda k: lxt[:, k * N:(k + 1) * N], convW["xz"]),
                 (lambda k: lht[:, k * N:(k + 1) * N], convW["hz"]),
                 (lambda k: lxt[:, k * N:(k + 1) * N], convW["xh"])], "zx")
            agh_z = ag_start(cvt_zx, 3 * KBL * N, "z")

            ag_r = ag_read(agh_r)
            pg_r = gate_pack(ag_r, 0, 2 * KBL * N, ag_r, KBL * N, 2 * KBL * N,
                             gateU["xr"], gateU["hr"], "r")
            r_s = gate_finish(pg_r, br_s, AF.Sigmoid, "r")

            # H*R -> local L(H*R).T blocks -> AllGather
            hr_s = fin.tile([N, CL], dt.bfloat16, name="hr_s")
            nc.vector.tensor_mul(hr_s, h_s, r_s)
            pl = ptr.tile([128, KBL * N], dt.float32, tag="tr")
            for kk in range(KBL):
                nc.tensor.matmul(pl[:, kk * N:(kk + 1) * N],
                                 hr_s[:, kk * 128:(kk + 1) * 128], m_t,
                                 start=True, stop=True)
            lrt_l = cvtp.tile([128, KBL * N], dt.bfloat16, name="lrt_l")
            nc.vector.tensor_copy(lrt_l, pl)
            agh_l = ag_start(lrt_l, KBL * N, "lrt")

            ag_z = ag_read(agh_z)
            ag_l = ag_read(agh_l)

            # ---------------- hh conv (needs AG_l) ------------------------
            cvt_hh, _, _ = conv_pack(
                [(lambda k: ag_l[:, (k // KBL) * (KBL * N)
                                 + (k % KBL) * N:
                                 (k // KBL) * (KBL * N)
                                 + (k % KBL + 1) * N], convW["hh"])], "hh")
            agh_hh = ag_start(cvt_hh, KBL * N, "hh")
            ag_hh = ag_read(agh_hh)

            # ---------------- gate h ------------------------------------
            pg_h = gate_pack(ag_z, 2 * KBL * N, 3 * KBL * N,
                             ag_hh, 0, KBL * N,
                             gateU["xh"], gateU["hh"], "h")
            ht_s = gate_finish(pg_h, bh_s, AF.Tanh, "h")

            # ---------------- gate z ------------------------------------
            pg_z = gate_pack(ag_z, 0, 3 * KBL * N, ag_z, KBL * N, 3 * KBL * N,
                             gateU["xz"], gateU["hz"], "z")
            z_s = gate_finish(pg_z, bz_s, AF.Sigmoid, "z")

            # -------- final combine: out = H + Z * (Ht - H) ---------------
            o_s = fin.tile([N, CL], dt.float32, name="o_s")
            nc.vector.tensor_sub(o_s, ht_s, h_s)
            nc.vector.tensor_mul(o_s, o_s, z_s)
            nc.vector.tensor_add(o_s, o_s, h_s)
            nc.scalar.dma_start(out[:, :], o_s)

    nc.compile()
    return nc


def _host_prep(X, edge_index, edge_weight, H, lambda_max,
               Wc_xz, Wc_hz, Wc_xr, Wc_hr, Wc_xh, Wc_hh,
               w_x_z, w_q_z, w_x_r, w_q_r, w_x_h, w_q_h,
               b_z, b_r, b_h):
    """Build per-core input maps."""
    import ml_dtypes
    f32 = np.float32
    bf16 = ml_dtypes.bfloat16
    f8 = ml_dtypes.float8_e3m4
    X = np.ascontiguousarray(np.asarray(X, f32))
    H = np.ascontiguousarray(np.asarray(H, f32))
    ei = np.asarray(edge_index)
    ew = np.asarray(edge_weight, np.float64)
    lam = float(np.asarray(lambda_max).reshape(-1)[0])

    src = ei[0].astype(np.int64)
    dst = ei[1].astype(np.int64)
    deg_s = np.bincount(src, weights=ew, minlength=N)[:N]
    deg_d = np.bincount(dst, weights=ew, minlength=N)[:N]
    dinv_s = np.where(deg_s > 0, 1.0 / np.sqrt(np.maximum(deg_s, 1e-12)), 0.0)
    dinv_d = np.where(deg_d > 0, 1.0 / np.sqrt(np.maximum(deg_d, 1e-12)), 0.0)
    norm = ew * dinv_s[src] * dinv_d[dst]
    A = np.zeros((N, N), np.float64)
    np.add.at(A, (dst, src), norm)
    Mp = (2.0 / lam - 1.0) * np.eye(N) - (2.0 / lam) * A   # LX = Mp @ T
    Mm = np.ascontiguousarray(Mp.T).astype(bf16)           # device rhs [j, i]
    I = np.eye(N, dtype=bf16)

    conv = {"xz": Wc_xz, "hz": Wc_hz, "xr": Wc_xr,
            "hr": Wc_hr, "xh": Wc_xh, "hh": Wc_hh}
    gate = {"xz": w_x_z, "hz": w_q_z, "xr": w_x_r,
            "hr": w_q_r, "xh": w_x_h, "hh": w_q_h}
    conv = {k: np.asarray(v, f32) for k, v in conv.items()}
    gate = {k: np.asarray(v, f32) for k, v in gate.items()}
    b_r = np.asarray(b_r, f32)
    b_z = np.asarray(b_z, f32)
    b_h = np.asarray(b_h, f32)

    def q8(w):
        q = np.clip(np.asarray(w, np.float64) * SCALE,
                    -15.5, 15.5).astype(f8)
        # pre-tile [C, CL] -> [128, KB*CL]: k-block k at cols k*CL
        return np.ascontiguousarray(
            q.reshape(KB, 128, CL).transpose(1, 0, 2).reshape(128, KB * CL))

    def tile_layout(lt):
        # LX.T [C, N] -> [128, 32*24] with block k at cols k*24
        return np.ascontiguousarray(
            lt.reshape(KB, 128, N).transpose(1, 0, 2).reshape(128, KB * N))

    LX = (Mp @ X.astype(np.float64))
    LH = (Mp @ H.astype(np.float64))
    common = {
        "Mm": Mm, "I24": I,
        "WarmI": np.zeros((128, 512), bf16),
        "LXT": tile_layout(LX.T).astype(bf16),
        "LHT": tile_layout(LH.T).astype(bf16),
    }
    in_maps = []
    for c in range(NCORES):
        sl = slice(c * CL, (c + 1) * CL)
        m = dict(common)
        for g in ("xz", "hz", "xr", "hr", "xh", "hh"):
            m[f"W{g}"] = np.ascontiguousarray(q8(conv[g][:, sl]))
            m[f"U{g}"] = np.ascontiguousarray(q8(gate[g][sl, :].T))
        m["BrS"] = np.ascontiguousarray(b_r[:, sl])
        m["BzS"] = np.ascontiguousarray(b_z[:, sl])
        m["BhS"] = np.ascontiguousarray(b_h[:, sl])
        m["HS"] = np.ascontiguousarray(H[:, sl])
        in_maps.append(m)
    return in_maps


def kernel(**inputs):
    global LAST_RESULT
    if "nc" not in _CACHE:
        _CACHE["nc"] = _build()
    nc = _CACHE["nc"]
    in_maps = _host_prep(**inputs)
    res = run_bass_kernel_spmd(nc, in_maps, list(range(NCORES)), **RUN_KWARGS)
    LAST_RESULT = res
    return np.concatenate(
        [np.asarray(res.results[c]["out"], np.float32)
         for c in range(NCORES)], axis=1)
